# revision 79
# baseline (speedup 1.0000x reference)
"""Causal self-attention (B=4, T=2048, E=1024, H=16, D=64) on 8 TRN2 NeuronCores.

Sharding: core = b*2 + g  (data parallel over batch b in 0..3, tensor parallel
over head-halves g in 0..1; 8 local heads per core, column-split QKV /
row-split out projection). Host sums the two partial out-projections per batch
and adds b_out.

Device kernel (per core). All matmuls run with fp16 operands (1 cycle/row on
the PE) accumulating in fp32 PSUM; activations/weights are pre-cast to fp16 on
the host so they DMA straight into their SBUF tiles:
  - qT/kT [128 = 2 heads x 64, T] per head-pair; v' [T, 8 x (64 v-dims + ones
    col)]; the ones column makes the attn@v matmul emit softmax denominators.
  - transposed-scores attention per (pair, t-block of 512): scoresT[s,t]
    chunks via row-tiled K=64 matmul pairs into a 2-bank PSUM tile, one exp
    per chunk on ACT (both heads, scale=1/8 folded in), causal diagonal via
    in-place [128,128] triangle multiplies on DVE, av accumulated over
    s-chunks with causal width narrowing.
  - PE kept dense (HAM warm) while ACT grinds exps: remaining qkv-projection
    groups and out-projection groups are fed through the PSUM slot freed by
    each block's normalization.
  - normalization off the PE path: denominator rows bounce through DRAM and
    partition-broadcast back by DMA, one-shot reciprocal_approx_fast on DVE,
    multiplied into the fp16 attention output.
  - lead-in: pair-0 q/k run as an e-major accumulation wave across 8 PSUM
    banks so the PE consumes each x chunk as its DMA lands; qkv/out-projection
    fill work is drip-fed one group per two attention chunks from a
    deadline-sorted queue; y is written back as fp16 and summed on the host.
"""
import numpy as np

B, T, E, H, D = 4, 2048, 1024, 16, 64
HL = H // 2           # local heads per core (8)
NP = HL // 2          # head pairs per core (4)
EL = HL * D           # local attn-out width (512)
VW = HL * (D + 1)     # v' width with ones columns (520)
NCORES = 8
NB = T // 512         # t-blocks (4)
NC = T // 128         # s-chunks (16)
NE = E // 128         # e-chunks (8)

_cache = {}


def _build_nc():
    import concourse.bacc as bacc
    import concourse.mybir as mybir
    from concourse.tile import TileContext

    F32 = mybir.dt.float32
    F16 = mybir.dt.float16
    EXP = mybir.ActivationFunctionType.Exp

    nc = bacc.Bacc(None, target_bir_lowering=False)
    xT = nc.dram_tensor("xT", [E, T], F16, kind="ExternalInput")
    wq0d = nc.dram_tensor("wq0d", [128, NE, 128], F16, kind="ExternalInput")
    wk0d = nc.dram_tensor("wk0d", [128, NE, 128], F16, kind="ExternalInput")
    wqkr = nc.dram_tensor("wqkr", [128, 6, NE, 128], F16, kind="ExternalInput")
    wv2d = nc.dram_tensor("wv2d", [2, 128, NE, VW // 2], F16, kind="ExternalInput")
    wod = nc.dram_tensor("wod", [128, NP, E], F16, kind="ExternalInput")
    rowsd = nc.dram_tensor("rowsd", [1, VW], F16, kind="ExternalInput")   # bv2
    bcold = nc.dram_tensor("bcold", [128, 2 * NP], F32, kind="ExternalInput")
    trid = nc.dram_tensor("trid", [128, 128], F16, kind="ExternalInput")
    y = nc.dram_tensor("y", [T, E], F16, kind="ExternalOutput")

    with TileContext(nc) as tc:
        with (
            tc.tile_pool(name="const", bufs=1) as cpool,
            tc.tile_pool(name="p_keep", bufs=1) as keep,
            tc.tile_pool(name="p_st", bufs=2) as st,
        ):
            # ---- long-lived fp16 tensors, DMA'd directly (priority order) ----
            HALF = VW // 2  # 260
            xt = [keep.tile([128, T], F16, name=f"xt{e}", tag=f"xt{e}") for e in range(NE)]
            wr = {}
            wr[(0, "q")] = keep.tile([128, NE, 128], F16, name="wq0", tag="wq0")
            wr[(0, "k")] = keep.tile([128, NE, 128], F16, name="wk0", tag="wk0")
            wr_rest = keep.tile([128, 6, NE, 128], F16, name="wrest", tag="wrest")
            for p in range(1, NP):
                for i, nm in enumerate(("q", "k")):
                    wr[(p, nm)] = wr_rest[:, 2 * (p - 1) + i]
            wv_r = [keep.tile([128, NE, HALF], F16, name=f"wv{h_}", tag=f"wv{h_}")
                    for h_ in range(2)]
            # priority: pair-0 q/k weights (scalar queue) race with x chunks
            # (sync); bulk weights follow x on the sync queue so they don't
            # steal HBM bandwidth from the critical path.
            nc.scalar.dma_start(out=wr[(0, "q")], in_=wq0d[:, :, :])
            nc.sync.dma_start(out=xt[0][:, 0:1024], in_=xT[0:128, 0:1024])
            nc.sync.dma_start(out=xt[0][:, 1024:2048], in_=xT[0:128, 1024:2048])
            for e in range(1, NE):
                nc.sync.dma_start(out=xt[e], in_=xT[e * 128:(e + 1) * 128, :])
            nc.scalar.dma_start(out=wr[(0, "k")], in_=wk0d[:, :, :])
            nc.sync.dma_start(out=wv_r[0], in_=wv2d[0])
            nc.sync.dma_start(out=wr_rest[:, 0:2], in_=wqkr[:, 0:2])
            nc.sync.dma_start(out=wv_r[1], in_=wv2d[1])
            nc.sync.dma_start(out=wr_rest[:, 2:6], in_=wqkr[:, 2:6])
            wo_r = keep.tile([128, NP, E], F16, name="wo_r")
            nc.sync.dma_start(out=wo_r, in_=wod[:, :, :])
            # ---- constants ----
            tri_sb = cpool.tile([128, 128], F16, name="tri_sb")
            nc.scalar.dma_start(out=tri_sb, in_=trid[:, :])
            bcol = cpool.tile([128, 2 * NP], F32, name="bcol")
            nc.scalar.dma_start(out=bcol, in_=bcold[:, :])
            ones_r = cpool.tile([1, 512], F16, name="ones_r")
            nc.vector.memset(ones_r, 1.0)
            bv_r = cpool.tile([1, VW], F16, name="bv_r")
            nc.scalar.dma_start(out=bv_r, in_=rowsd[:, :])
            # preload the ACT exp table during the lead-in
            warm = cpool.tile([1, 16], F32, name="warm")
            nc.scalar.activation(warm, tri_sb[0:1, 0:16], EXP, scale=0.125)
            qt = [keep.tile([128, T], F16, name=f"qt{p}", tag=f"qt{p}") for p in range(NP)]
            kt = [keep.tile([128, T], F16, name=f"kt{p}", tag=f"kt{p}") for p in range(NP)]
            vt = [keep.tile([128, VW], F16, name=f"vt{t_}", tag=f"vt{t_}") for t_ in range(NC)]
            ao = [keep.tile([128, T], F16, name=f"ao{p}", tag=f"ao{p}") for p in range(NP)]

            # ---- lead-in: pair-0 q/k via an e-major accumulation wave so the
            # PE consumes each x chunk as it lands (8 open PSUM groups) ----
            with tc.tile_pool(name="psL", bufs=1, space="PSUM") as psL:
                lead = []
                for i, (nm, tb) in enumerate(
                        [(nm, tb) for tb in range(NB) for nm in ("q", "k")]):
                    lead.append((nm, tb, psL.tile(
                        [128, 512], F32, name=f"pl{i}", tag=f"g{i}")))
                for e in range(NE - 1):
                    for nm, tb, ps in lead:
                        nc.tensor.matmul(
                            ps, wr[(0, nm)][:, e, :],
                            xt[e][:, tb * 512:(tb + 1) * 512],
                            start=(e == 0), stop=False,
                        )
                # last e-chunk per group, bias-add right after so the adds
                # overlap the remaining groups' matmuls (tb=0 first so the
                # first attention block unblocks early)
                for nm, tb, ps in lead:
                    nc.tensor.matmul(
                        ps, wr[(0, nm)][:, NE - 1, :],
                        xt[NE - 1][:, tb * 512:(tb + 1) * 512],
                        start=False, stop=True,
                    )
                    dst = qt[0] if nm == "q" else kt[0]
                    col = 0 if nm == "q" else 1
                    nc.scalar.add(dst[:, tb * 512:(tb + 1) * 512], ps,
                                  bcol[:, col:col + 1])

            with (
                tc.tile_pool(name="p_att", bufs=6) as att,
                tc.tile_pool(name="p_nrm", bufs=3) as nrm,
                tc.tile_pool(name="p_dr", bufs=4, space="DRAM") as drp,
                tc.tile_pool(name="psA", bufs=2, space="PSUM") as psA,
            ):
                def emit_qk_group(p, nm, tb, on_act=False):
                    dst = qt[p] if nm == "q" else kt[p]
                    ps = psA.tile([128, 512], F32, name=f"ps{nm}_{p}_{tb}", tag="av")
                    for e in range(NE):
                        nc.tensor.matmul(
                            ps, wr[(p, nm)][:, e, :], xt[e][:, tb * 512:(tb + 1) * 512],
                            start=(e == 0), stop=(e == NE - 1),
                        )
                    col = 2 * p + (0 if nm == "q" else 1)
                    dsl = dst[:, tb * 512:(tb + 1) * 512]
                    if on_act:
                        nc.scalar.add(dsl, ps, bcol[:, col:col + 1])
                    else:
                        nc.vector.tensor_scalar_add(dsl, ps, bcol[:, col:col + 1])

                def emit_v_group(tt, h_):
                    ps = psA.tile([128, HALF], F32, name=f"psv_{tt}_{h_}", tag="av")
                    for e in range(NE):
                        nc.tensor.matmul(
                            ps, xt[e][:, tt * 128:(tt + 1) * 128], wv_r[h_][:, e, :],
                            start=(e == 0), stop=False,
                        )
                    nc.tensor.matmul(
                        ps, ones_r[:, 0:128], bv_r[:, h_ * HALF:(h_ + 1) * HALF],
                        start=False, stop=True,
                    )
                    dsl = vt[tt][:, h_ * HALF:(h_ + 1) * HALF]
                    nc.vector.tensor_copy(dsl, ps)

                def emit_scores(p, tb, c):
                    j = c - 4 * tb
                    lo = 128 * j if j >= 0 else 0
                    sp = psA.tile([128, 1024], F32, name=f"s_{p}_{tb}_{c}", tag="s")
                    nc.tensor.matmul(
                        sp[:, lo:512], kt[p][0:64, c * 128:(c + 1) * 128],
                        qt[p][0:64, tb * 512 + lo:(tb + 1) * 512],
                        start=True, stop=True, tile_position=(0, 0),
                    )
                    nc.tensor.matmul(
                        sp[:, 512 + lo:1024], kt[p][64:128, c * 128:(c + 1) * 128],
                        qt[p][64:128, tb * 512 + lo:(tb + 1) * 512],
                        start=True, stop=True, tile_position=(64, 0),
                    )
                    return sp, lo, j

                def emit_expav(p, tb, c, sc, av0, av1, nch):
                    sp, lo, j = sc
                    ep = att.tile([128, 1024], F16, name=f"e_{p}_{tb}_{c}", tag="ep")
                    if j < 0:
                        nc.scalar.activation(ep, sp, EXP, scale=0.125)
                    else:
                        spv = sp[:, :].rearrange("q (h t) -> q h t", h=2)
                        epv = ep[:, :].rearrange("q (h t) -> q h t", h=2)
                        nc.scalar.activation(epv[:, :, lo:512], spv[:, :, lo:512],
                                             EXP, scale=0.125)
                        for h in range(2):
                            nc.vector.tensor_mul(
                                epv[:, h, lo:lo + 128], epv[:, h, lo:lo + 128], tri_sb)
                    for h, av in ((0, av0), (1, av1)):
                        vcol = 65 * (2 * p + h)
                        nc.tensor.matmul(
                            av[:, lo:512], vt[c][:, vcol:vcol + 65],
                            ep[:, 512 * h + lo:512 * h + 512],
                            start=(c == 0), stop=(c == nch - 1),
                        )

                def norm_thunks(p, tb, avp, fast=False):
                    # normalization as ~0.7us DVE pieces, dripped one per chunk
                    # so diagonal tri-multiplies interleave instead of queueing
                    # behind a 4us burst. fast=True (final block): the bounce
                    # DMAs ride the HWDGE scalar queue (~0.6us first byte)
                    # instead of the gpsimd SWDGE queue (~1-2us).
                    dma = nc.scalar if fast else nc.gpsimd
                    av0 = avp[:, 0:512]
                    av1 = avp[:, 512:1024]
                    dh = nrm.tile([65, 512], F32, name=f"dh_{p}_{tb}", tag="dh")
                    tmpu = nrm.tile([65, 512], F32, name=f"tu_{p}_{tb}", tag="tu")
                    ao_raw = nrm.tile([128, 512], F32, name=f"ar_{p}_{tb}", tag="ar")
                    dscr = drp.tile([2, 512], F32, name=f"ds_{p}_{tb}", tag="ds")
                    bc = nrm.tile([128, 512], F32, name=f"bc_{p}_{tb}", tag="bc")
                    rc = nrm.tile([128, 512], F32, name=f"rc_{p}_{tb}", tag="rc")
                    tbsl = slice(tb * 512, (tb + 1) * 512)

                    def s1():
                        nc.vector.tensor_copy(dh[64:65, :], av0[64:65, :])
                        dma.dma_start(out=dscr[0:1, :], in_=dh[64:65, :])

                    def s2():
                        nc.vector.tensor_copy(tmpu, av1[0:65, :])
                        dma.dma_start(out=dscr[1:2, :], in_=tmpu[64:65, :])
                        dma.dma_start(out=ao_raw[64:128, :], in_=tmpu[0:64, :])
                        dma.dma_start(
                            out=bc[0:64, :],
                            in_=dscr[0:1, :].partition_broadcast(64))
                        dma.dma_start(
                            out=bc[64:128, :],
                            in_=dscr[1:2, :].partition_broadcast(64))

                    def s3():
                        nc.vector.tensor_copy(ao_raw[0:64, :], av0[0:64, :])

                    def s4():
                        nc.vector.reciprocal_approx_fast(out=rc, in_=bc)

                    def s5():
                        nc.vector.tensor_mul(ao[p][:, tbsl], ao_raw, rc)

                    return [s1, s2, s3, s4, s5]

                def emit_norm(p, tb, avp, fast=False):
                    for s in norm_thunks(p, tb, avp, fast=fast):
                        s()

                def emit_expav_split(p, tb, c, sc, avp, nch):
                    # final block: av accumulation split at the t-midpoint with
                    # each t-half in its OWN PSUM bank (one open accumulation
                    # group per bank). Bank A (cols 0:512) = both heads'
                    # t[0:256), closes at chunk 13; bank B (cols 512:1024) =
                    # t[256:512), closes at the last chunk.
                    sp, lo, j = sc
                    ep = att.tile([128, 1024], F16, name=f"e_{p}_{tb}_{c}", tag="ep")
                    if j < 0:
                        nc.scalar.activation(ep, sp, EXP, scale=0.125)
                    else:
                        spv = sp[:, :].rearrange("q (h t) -> q h t", h=2)
                        epv = ep[:, :].rearrange("q (h t) -> q h t", h=2)
                        nc.scalar.activation(epv[:, :, lo:512], spv[:, :, lo:512],
                                             EXP, scale=0.125)
                        for h in range(2):
                            nc.vector.tensor_mul(
                                epv[:, h, lo:lo + 128], epv[:, h, lo:lo + 128], tri_sb)
                    for h in range(2):
                        vcol = 65 * (2 * p + h)
                        if lo < 256:
                            nc.tensor.matmul(
                                avp[:, 256 * h + lo:256 * h + 256],
                                vt[c][:, vcol:vcol + 65],
                                ep[:, 512 * h + lo:512 * h + 256],
                                start=(c == 0), stop=(c == 13),
                            )
                        blo = max(lo, 256)
                        nc.tensor.matmul(
                            avp[:, 512 + 256 * h + blo - 256:512 + 256 * h + 256],
                            vt[c][:, vcol:vcol + 65],
                            ep[:, 512 * h + blo:512 * h + 512],
                            start=(c == 0), stop=(c == nch - 1),
                        )

                def emit_norm_half(p, tb, avp, half, fast=False):
                    # bank-local layout: h0 at [ob:ob+256], h1 at [ob+256:ob+512]
                    dma = nc.scalar if fast else nc.gpsimd
                    ob = 512 * half
                    W = 256
                    dh = nrm.tile([65, W], F32, name=f"dhh_{half}", tag="dhh")
                    tmpu = nrm.tile([65, W], F32, name=f"tuh_{half}", tag="tuh")
                    ao_raw = nrm.tile([128, W], F32, name=f"arh_{half}", tag="arh")
                    nc.vector.tensor_copy(dh[64:65, :], avp[64:65, ob:ob + W])
                    nc.vector.tensor_copy(tmpu, avp[0:65, ob + 256:ob + 512])
                    dscr = drp.tile([2, W], F32, name=f"dsh_{half}", tag="dsh")
                    dma.dma_start(out=dscr[0:1, :], in_=dh[64:65, :])
                    dma.dma_start(out=dscr[1:2, :], in_=tmpu[64:65, :])
                    nc.vector.tensor_copy(ao_raw[0:64, :], avp[0:64, ob:ob + W])
                    dma.dma_start(out=ao_raw[64:128, :], in_=tmpu[0:64, :])
                    bc = nrm.tile([128, W], F32, name=f"bch_{half}", tag="bch")
                    dma.dma_start(
                        out=bc[0:64, :], in_=dscr[0:1, :].partition_broadcast(64))
                    dma.dma_start(
                        out=bc[64:128, :], in_=dscr[1:2, :].partition_broadcast(64))
                    rc = nrm.tile([128, W], F32, name=f"rch_{half}", tag="rch")
                    nc.vector.reciprocal_approx_fast(out=rc, in_=bc)
                    tsl = slice(tb * 512 + 256 * half, tb * 512 + 256 * half + W)
                    nc.vector.tensor_mul(ao[p][:, tsl], ao_raw, rc)

                def emit_proj_group(tt, eb, tag="av", on_act=False):
                    ps = psA.tile([128, 512], F32, name=f"py_{tt}_{eb}", tag=tag)
                    for pp in range(NP):
                        nc.tensor.matmul(
                            ps, ao[pp][:, tt * 128:(tt + 1) * 128],
                            wo_r[:, pp, eb * 512:(eb + 1) * 512],
                            start=(pp == 0), stop=(pp == NP - 1),
                        )
                    ys = st.tile([128, 512], F16, name=f"ys_{tt}_{eb}", tag="ys", bufs=8)
                    if on_act:
                        nc.scalar.copy(ys, ps)
                    else:
                        nc.vector.tensor_copy(ys, ps)
                    nc.sync.dma_start(
                        out=y[tt * 128:(tt + 1) * 128, eb * 512:(eb + 1) * 512], in_=ys)

                # ---- v half 0 for the first t-block (pairs 0/1), plus two
                # extra groups to bridge the PE gap while ACT drains the
                # lead-in bias adds (keeps the HAM clock-gate warm) ----
                for tt in range(6):
                    emit_v_group(tt, 0)

                blocks = [(p, tb) for p in range(NP) for tb in range(NB)]
                bidx = {b: i for i, b in enumerate(blocks)}

                # window queue with deadlines: item = (deadline, kind, args);
                # deadline i = must be emitted before block i+1 starts.
                wq_items = []
                for tt in range(6, NC):
                    wq_items.append((bidx[(0, tt // 4)] - 1, "v", (tt, 0)))
                for tt in range(NC):
                    wq_items.append((bidx[(2, tt // 4)] - 1, "v", (tt, 1)))
                for pp in (1, 2, 3):
                    for tb in range(NB):
                        for nm in ("q", "k"):
                            wq_items.append(
                                (bidx[(pp, tb)] - 1, "qk", (pp, nm, tb)))
                wq_items.sort(key=lambda it: it[0])
                NODL = 999

                def emit_item(kind, args):
                    if kind == "qk":
                        emit_qk_group(*args)
                    elif kind == "v":
                        emit_v_group(*args)
                    else:
                        emit_proj_group(*args)

                def pop_window(n):
                    for _ in range(n):
                        if wq_items:
                            _, kind, args = wq_items.pop(0)
                            emit_item(kind, args)

                def drain_due(key):
                    while wq_items and wq_items[0][0] <= key:
                        _, kind, args = wq_items.pop(0)
                        emit_item(kind, args)

                pending = None
                pre_sc = None
                nsteps = []
                nmeta = None

                def norm_step():
                    nonlocal nsteps, nmeta
                    if nsteps:
                        nsteps.pop(0)()
                        if not nsteps and nmeta is not None:
                            np_, ntb_ = nmeta
                            nmeta = None
                            if np_ == NP - 1:
                                wq_items.extend(
                                    (NODL, "proj", (tt, eb))
                                    for tt in range(4 * ntb_, 4 * ntb_ + 4)
                                    for eb in range(2))

                for bi, (p, tb) in enumerate(blocks):
                    drain_due(bi)
                    nch = 4 * (tb + 1)
                    avp = psA.tile([65, 1024], F32, name=f"avp_{p}_{tb}", tag="av")
                    av0 = avp[:, 0:512]
                    av1 = avp[:, 512:1024]
                    sc = pre_sc if pre_sc is not None else emit_scores(p, tb, 0)
                    pre_sc = None
                    # emit the previous block's norm early (frees its PSUM slot
                    # for the interleaved groups); for short all-diagonal blocks
                    # wait until a couple of tri-multiplies are queued on DVE.
                    norm_c = min(3, nch - 1)
                    for c in range(nch):
                        if c + 1 < nch:
                            sc_next = emit_scores(p, tb, c + 1)
                        elif bi + 1 < len(blocks):
                            # cross-block lookahead: next block's first scores
                            pre_sc = emit_scores(blocks[bi + 1][0], blocks[bi + 1][1], 0)
                            sc_next = None
                        else:
                            sc_next = None
                        emit_expav(p, tb, c, sc, av0, av1, nch)
                        sc = sc_next
                        if c == norm_c and pending is not None:
                            nsteps = norm_thunks(*pending)
                            nmeta = (pending[0], pending[1])
                            pending = None
                        norm_step()
                        if c % 2 == 1:
                            pop_window(1)
                    while nsteps:
                        norm_step()
                    pending = (p, tb, avp)
                emit_norm(*pending, fast=True)
                wq_items.extend((NODL, "proj", (tt, eb))
                                for tt in range(12, 16) for eb in range(2))
                i = 0
                while wq_items:
                    _, kind, args = wq_items.pop(0)
                    if kind == "proj":
                        emit_proj_group(*args, tag=("s" if i % 2 else "av"), on_act=True)
                        i += 1
                    else:
                        emit_item(kind, args)

    nc.compile()
    return nc


def get_nc():
    if "nc" not in _cache:
        _cache["nc"] = _build_nc()
    return _cache["nc"]


def make_in_maps(x, w_qkv, b_qkv, w_out, b_out):
    """Per-core input dicts. Core = b*2 + g."""
    x = np.asarray(x, dtype=np.float32)
    w_qkv = np.asarray(w_qkv, dtype=np.float32)
    b_qkv = np.asarray(b_qkv, dtype=np.float32)
    w_out = np.asarray(w_out, dtype=np.float32)

    wq_full, wk_full, wv_full = w_qkv[:, 0:E], w_qkv[:, E:2 * E], w_qkv[:, 2 * E:3 * E]
    bq_full, bk_full, bv_full = b_qkv[0:E], b_qkv[E:2 * E], b_qkv[2 * E:3 * E]

    idx = np.arange(128)
    tri = (idx[:, None] <= idx[None, :]).astype(np.float16)  # tri[s,t]=1 iff s<=t

    in_maps = []
    for core in range(NCORES):
        b, g = core // 2, core % 2
        h0 = g * HL
        cols = slice(h0 * D, (h0 + HL) * D)
        wq_l = wq_full[:, cols]
        wk_l = wk_full[:, cols]
        wv_l = wv_full[:, cols]
        bq_l = bq_full[cols]
        bk_l = bk_full[cols]
        bv_l = bv_full[cols]

        wqk_s = np.empty((2 * NP, 128, NE, 128), dtype=np.float16)
        for p in range(NP):
            wqk_s[2 * p] = wq_l[:, p * 128:(p + 1) * 128].reshape(NE, 128, 128).transpose(1, 0, 2)
            wqk_s[2 * p + 1] = wk_l[:, p * 128:(p + 1) * 128].reshape(NE, 128, 128).transpose(1, 0, 2)

        wv2 = np.zeros((E, VW), dtype=np.float16)
        bv2 = np.zeros((1, VW), dtype=np.float16)
        for h in range(HL):
            wv2[:, h * 65:h * 65 + 64] = wv_l[:, h * 64:(h + 1) * 64].astype(np.float16)
            bv2[0, h * 65:h * 65 + 64] = bv_l[h * 64:(h + 1) * 64].astype(np.float16)
            bv2[0, h * 65 + 64] = 1.0

        bcol = np.zeros((128, 2 * NP), dtype=np.float32)
        for p in range(NP):
            bcol[:, 2 * p] = bq_l[p * 128:(p + 1) * 128]
            bcol[:, 2 * p + 1] = bk_l[p * 128:(p + 1) * 128]

        wv2d = wv2.reshape(NE, 128, 2, VW // 2).transpose(2, 1, 0, 3)
        wo_l = w_out[g * EL:(g + 1) * EL, :].astype(np.float16)
        wo2 = wo_l.reshape(NP, 128, E).transpose(1, 0, 2)
        in_maps.append({
            "xT": np.ascontiguousarray(x[b].T.astype(np.float16)),
            "wq0d": np.ascontiguousarray(wqk_s[0]),
            "wk0d": np.ascontiguousarray(wqk_s[1]),
            "wqkr": np.ascontiguousarray(wqk_s[2:8].transpose(1, 0, 2, 3)),
            "wv2d": np.ascontiguousarray(wv2d),
            "wod": np.ascontiguousarray(wo2),
            "rowsd": bv2,
            "bcold": bcol,
            "trid": tri,
        })
    return in_maps


def gather_output(results, b_out):
    out = np.empty((B, T, E), dtype=np.float32)
    b_out = np.asarray(b_out, dtype=np.float32)
    for b in range(B):
        out[b] = (results[2 * b]["y"].astype(np.float32)
                  + results[2 * b + 1]["y"].astype(np.float32) + b_out[None, :])
    return out


def kernel(x, w_qkv, b_qkv, w_out, b_out):
    from concourse.bass_utils import run_bass_kernel_spmd

    nc = get_nc()
    in_maps = make_in_maps(x, w_qkv, b_qkv, w_out, b_out)
    r = run_bass_kernel_spmd(nc, in_maps, core_ids=list(range(NCORES)))
    return gather_output(r.results, np.asarray(b_out, dtype=np.float32))


# revision 80
# speedup vs baseline: 1.0041x; 1.0041x over previous
"""Causal self-attention (B=4, T=2048, E=1024, H=16, D=64) on 8 TRN2 NeuronCores.

Sharding: core = b*2 + g  (data parallel over batch b in 0..3, tensor parallel
over head-halves g in 0..1; 8 local heads per core, column-split QKV /
row-split out projection). Host sums the two partial out-projections per batch
and adds b_out.

Device kernel (per core). All matmuls run with fp16 operands (1 cycle/row on
the PE) accumulating in fp32 PSUM; activations/weights are pre-cast to fp16 on
the host so they DMA straight into their SBUF tiles:
  - qT/kT [128 = 2 heads x 64, T] per head-pair; v' [T, 8 x (64 v-dims + ones
    col)]; the ones column makes the attn@v matmul emit softmax denominators.
  - transposed-scores attention per (pair, t-block of 512): scoresT[s,t]
    chunks via row-tiled K=64 matmul pairs into a 2-bank PSUM tile, one exp
    per chunk on ACT (both heads, scale=1/8 folded in), causal diagonal via
    in-place [128,128] triangle multiplies on DVE, av accumulated over
    s-chunks with causal width narrowing.
  - PE kept dense (HAM warm) while ACT grinds exps: remaining qkv-projection
    groups and out-projection groups are fed through the PSUM slot freed by
    each block's normalization.
  - normalization off the PE path: denominator rows bounce through DRAM and
    partition-broadcast back by DMA, one-shot reciprocal_approx_fast on DVE,
    multiplied into the fp16 attention output.
  - lead-in: pair-0 q/k run as an e-major accumulation wave across 8 PSUM
    banks so the PE consumes each x chunk as its DMA lands; qkv/out-projection
    fill work is drip-fed one group per two attention chunks from a
    deadline-sorted queue; y is written back as fp16 and summed on the host.
"""
import numpy as np

B, T, E, H, D = 4, 2048, 1024, 16, 64
HL = H // 2           # local heads per core (8)
NP = HL // 2          # head pairs per core (4)
EL = HL * D           # local attn-out width (512)
VW = HL * (D + 1)     # v' width with ones columns (520)
NCORES = 8
NB = T // 512         # t-blocks (4)
NC = T // 128         # s-chunks (16)
NE = E // 128         # e-chunks (8)

_cache = {}


def _build_nc():
    import concourse.bacc as bacc
    import concourse.mybir as mybir
    from concourse.tile import TileContext

    F32 = mybir.dt.float32
    F16 = mybir.dt.float16
    EXP = mybir.ActivationFunctionType.Exp

    nc = bacc.Bacc(None, target_bir_lowering=False)
    xT = nc.dram_tensor("xT", [E, T], F16, kind="ExternalInput")
    wq0d = nc.dram_tensor("wq0d", [128, NE, 128], F16, kind="ExternalInput")
    wk0d = nc.dram_tensor("wk0d", [128, NE, 128], F16, kind="ExternalInput")
    wqkr = nc.dram_tensor("wqkr", [128, 6, NE, 128], F16, kind="ExternalInput")
    wv2d = nc.dram_tensor("wv2d", [2, 128, NE, VW // 2], F16, kind="ExternalInput")
    wod = nc.dram_tensor("wod", [128, NP, E], F16, kind="ExternalInput")
    rowsd = nc.dram_tensor("rowsd", [1, VW], F16, kind="ExternalInput")   # bv2
    bcold = nc.dram_tensor("bcold", [128, 2 * NP], F32, kind="ExternalInput")
    trid = nc.dram_tensor("trid", [128, 128], F16, kind="ExternalInput")
    y = nc.dram_tensor("y", [T, E], F16, kind="ExternalOutput")

    with TileContext(nc) as tc:
        with (
            tc.tile_pool(name="const", bufs=1) as cpool,
            tc.tile_pool(name="p_keep", bufs=1) as keep,
            tc.tile_pool(name="p_st", bufs=2) as st,
        ):
            # ---- long-lived fp16 tensors, DMA'd directly (priority order) ----
            HALF = VW // 2  # 260
            xt = [keep.tile([128, T], F16, name=f"xt{e}", tag=f"xt{e}") for e in range(NE)]
            wr = {}
            wr[(0, "q")] = keep.tile([128, NE, 128], F16, name="wq0", tag="wq0")
            wr[(0, "k")] = keep.tile([128, NE, 128], F16, name="wk0", tag="wk0")
            wr_rest = keep.tile([128, 6, NE, 128], F16, name="wrest", tag="wrest")
            for p in range(1, NP):
                for i, nm in enumerate(("q", "k")):
                    wr[(p, nm)] = wr_rest[:, 2 * (p - 1) + i]
            wv_r = [keep.tile([128, NE, HALF], F16, name=f"wv{h_}", tag=f"wv{h_}")
                    for h_ in range(2)]
            # priority: pair-0 q/k weights (scalar queue) race with x chunks
            # (sync); bulk weights follow x on the sync queue so they don't
            # steal HBM bandwidth from the critical path.
            nc.scalar.dma_start(out=wr[(0, "q")], in_=wq0d[:, :, :])
            nc.sync.dma_start(out=xt[0][:, 0:1024], in_=xT[0:128, 0:1024])
            nc.sync.dma_start(out=xt[0][:, 1024:2048], in_=xT[0:128, 1024:2048])
            for e in range(1, NE):
                nc.sync.dma_start(out=xt[e], in_=xT[e * 128:(e + 1) * 128, :])
            nc.scalar.dma_start(out=wr[(0, "k")], in_=wk0d[:, :, :])
            nc.sync.dma_start(out=wv_r[0], in_=wv2d[0])
            nc.sync.dma_start(out=wr_rest[:, 0:2], in_=wqkr[:, 0:2])
            nc.sync.dma_start(out=wv_r[1], in_=wv2d[1])
            nc.sync.dma_start(out=wr_rest[:, 2:6], in_=wqkr[:, 2:6])
            wo_r = keep.tile([128, NP, E], F16, name="wo_r")
            nc.sync.dma_start(out=wo_r, in_=wod[:, :, :])
            # ---- constants ----
            tri_sb = cpool.tile([128, 128], F16, name="tri_sb")
            nc.scalar.dma_start(out=tri_sb, in_=trid[:, :])
            bcol = cpool.tile([128, 2 * NP], F32, name="bcol")
            nc.scalar.dma_start(out=bcol, in_=bcold[:, :])
            ones_r = cpool.tile([1, 512], F16, name="ones_r")
            nc.vector.memset(ones_r, 1.0)
            bv_r = cpool.tile([1, VW], F16, name="bv_r")
            nc.scalar.dma_start(out=bv_r, in_=rowsd[:, :])
            # preload the ACT exp table during the lead-in
            warm = cpool.tile([1, 16], F32, name="warm")
            nc.scalar.activation(warm, tri_sb[0:1, 0:16], EXP, scale=0.125)
            qt = [keep.tile([128, T], F16, name=f"qt{p}", tag=f"qt{p}") for p in range(NP)]
            kt = [keep.tile([128, T], F16, name=f"kt{p}", tag=f"kt{p}") for p in range(NP)]
            vt = [keep.tile([128, VW], F16, name=f"vt{t_}", tag=f"vt{t_}") for t_ in range(NC)]
            ao = [keep.tile([128, T], F16, name=f"ao{p}", tag=f"ao{p}") for p in range(NP)]

            # ---- lead-in: pair-0 q/k via an e-major accumulation wave so the
            # PE consumes each x chunk as it lands (8 open PSUM groups) ----
            with tc.tile_pool(name="psL", bufs=1, space="PSUM") as psL:
                lead = []
                for i, (nm, tb) in enumerate(
                        [(nm, tb) for tb in range(NB) for nm in ("q", "k")]):
                    lead.append((nm, tb, psL.tile(
                        [128, 512], F32, name=f"pl{i}", tag=f"g{i}")))
                for e in range(NE - 1):
                    for nm, tb, ps in lead:
                        nc.tensor.matmul(
                            ps, wr[(0, nm)][:, e, :],
                            xt[e][:, tb * 512:(tb + 1) * 512],
                            start=(e == 0), stop=False,
                        )
                # last e-chunk per group, bias-add right after so the adds
                # overlap the remaining groups' matmuls (tb=0 first so the
                # first attention block unblocks early)
                for nm, tb, ps in lead:
                    nc.tensor.matmul(
                        ps, wr[(0, nm)][:, NE - 1, :],
                        xt[NE - 1][:, tb * 512:(tb + 1) * 512],
                        start=False, stop=True,
                    )
                    dst = qt[0] if nm == "q" else kt[0]
                    col = 0 if nm == "q" else 1
                    nc.scalar.add(dst[:, tb * 512:(tb + 1) * 512], ps,
                                  bcol[:, col:col + 1])

            with (
                tc.tile_pool(name="p_att", bufs=6) as att,
                tc.tile_pool(name="p_nrm", bufs=3) as nrm,
                tc.tile_pool(name="p_dr", bufs=2, space="DRAM") as drp,
                tc.tile_pool(name="psA", bufs=2, space="PSUM") as psA,
            ):
                def emit_qk_group(p, nm, tb, on_act=False):
                    dst = qt[p] if nm == "q" else kt[p]
                    ps = psA.tile([128, 512], F32, name=f"ps{nm}_{p}_{tb}", tag="av")
                    for e in range(NE):
                        nc.tensor.matmul(
                            ps, wr[(p, nm)][:, e, :], xt[e][:, tb * 512:(tb + 1) * 512],
                            start=(e == 0), stop=(e == NE - 1),
                        )
                    col = 2 * p + (0 if nm == "q" else 1)
                    dsl = dst[:, tb * 512:(tb + 1) * 512]
                    if on_act:
                        nc.scalar.add(dsl, ps, bcol[:, col:col + 1])
                    else:
                        nc.vector.tensor_scalar_add(dsl, ps, bcol[:, col:col + 1])

                def emit_v_group(tt, h_):
                    ps = psA.tile([128, HALF], F32, name=f"psv_{tt}_{h_}", tag="av")
                    for e in range(NE):
                        nc.tensor.matmul(
                            ps, xt[e][:, tt * 128:(tt + 1) * 128], wv_r[h_][:, e, :],
                            start=(e == 0), stop=False,
                        )
                    nc.tensor.matmul(
                        ps, ones_r[:, 0:128], bv_r[:, h_ * HALF:(h_ + 1) * HALF],
                        start=False, stop=True,
                    )
                    dsl = vt[tt][:, h_ * HALF:(h_ + 1) * HALF]
                    nc.vector.tensor_copy(dsl, ps)

                def emit_scores(p, tb, c):
                    j = c - 4 * tb
                    lo = 128 * j if j >= 0 else 0
                    sp = psA.tile([128, 1024], F32, name=f"s_{p}_{tb}_{c}", tag="s")
                    nc.tensor.matmul(
                        sp[:, lo:512], kt[p][0:64, c * 128:(c + 1) * 128],
                        qt[p][0:64, tb * 512 + lo:(tb + 1) * 512],
                        start=True, stop=True, tile_position=(0, 0),
                    )
                    nc.tensor.matmul(
                        sp[:, 512 + lo:1024], kt[p][64:128, c * 128:(c + 1) * 128],
                        qt[p][64:128, tb * 512 + lo:(tb + 1) * 512],
                        start=True, stop=True, tile_position=(64, 0),
                    )
                    return sp, lo, j

                def emit_expav(p, tb, c, sc, av0, av1, nch):
                    sp, lo, j = sc
                    ep = att.tile([128, 1024], F16, name=f"e_{p}_{tb}_{c}", tag="ep")
                    if j < 0:
                        nc.scalar.activation(ep, sp, EXP, scale=0.125)
                    else:
                        spv = sp[:, :].rearrange("q (h t) -> q h t", h=2)
                        epv = ep[:, :].rearrange("q (h t) -> q h t", h=2)
                        nc.scalar.activation(epv[:, :, lo:512], spv[:, :, lo:512],
                                             EXP, scale=0.125)
                        for h in range(2):
                            nc.vector.tensor_mul(
                                epv[:, h, lo:lo + 128], epv[:, h, lo:lo + 128], tri_sb)
                    for h, av in ((0, av0), (1, av1)):
                        vcol = 65 * (2 * p + h)
                        nc.tensor.matmul(
                            av[:, lo:512], vt[c][:, vcol:vcol + 65],
                            ep[:, 512 * h + lo:512 * h + 512],
                            start=(c == 0), stop=(c == nch - 1),
                        )

                def norm_thunks(p, tb, avp, fast=False):
                    # normalization as ~0.7us DVE pieces, dripped one per chunk
                    # so diagonal tri-multiplies interleave instead of queueing
                    # behind a 4us burst. fast=True (final block): the bounce
                    # DMAs ride the HWDGE scalar queue (~0.6us first byte)
                    # instead of the gpsimd SWDGE queue (~1-2us).
                    dma = nc.scalar if fast else nc.gpsimd
                    av0 = avp[:, 0:512]
                    av1 = avp[:, 512:1024]
                    dh = nrm.tile([65, 512], F32, name=f"dh_{p}_{tb}", tag="dh")
                    tmpu = nrm.tile([65, 512], F32, name=f"tu_{p}_{tb}", tag="tu")
                    ao_raw = nrm.tile([128, 512], F32, name=f"ar_{p}_{tb}", tag="ar")
                    dscr = drp.tile([2, 512], F32, name=f"ds_{p}_{tb}", tag="ds")
                    bc = nrm.tile([128, 512], F32, name=f"bc_{p}_{tb}", tag="bc")
                    rc = nrm.tile([128, 512], F32, name=f"rc_{p}_{tb}", tag="rc")
                    tbsl = slice(tb * 512, (tb + 1) * 512)

                    def s1():
                        nc.vector.tensor_copy(dh[64:65, :], av0[64:65, :])
                        dma.dma_start(out=dscr[0:1, :], in_=dh[64:65, :])

                    def s2():
                        nc.vector.tensor_copy(tmpu, av1[0:65, :])
                        dma.dma_start(out=dscr[1:2, :], in_=tmpu[64:65, :])
                        dma.dma_start(out=ao_raw[64:128, :], in_=tmpu[0:64, :])
                        dma.dma_start(
                            out=bc[0:64, :],
                            in_=dscr[0:1, :].partition_broadcast(64))
                        dma.dma_start(
                            out=bc[64:128, :],
                            in_=dscr[1:2, :].partition_broadcast(64))

                    def s3():
                        nc.vector.tensor_copy(ao_raw[0:64, :], av0[0:64, :])

                    def s4():
                        nc.vector.reciprocal_approx_fast(out=rc, in_=bc)

                    def s5():
                        nc.vector.tensor_mul(ao[p][:, tbsl], ao_raw, rc)

                    return [s1, s2, s3, s4, s5]

                def emit_norm(p, tb, avp, fast=False):
                    for s in norm_thunks(p, tb, avp, fast=fast):
                        s()

                def emit_expav_split(p, tb, c, sc, avp, nch):
                    # final block: av accumulation split at the t-midpoint with
                    # each t-half in its OWN PSUM bank (one open accumulation
                    # group per bank). Bank A (cols 0:512) = both heads'
                    # t[0:256), closes at chunk 13; bank B (cols 512:1024) =
                    # t[256:512), closes at the last chunk.
                    sp, lo, j = sc
                    ep = att.tile([128, 1024], F16, name=f"e_{p}_{tb}_{c}", tag="ep")
                    if j < 0:
                        nc.scalar.activation(ep, sp, EXP, scale=0.125)
                    else:
                        spv = sp[:, :].rearrange("q (h t) -> q h t", h=2)
                        epv = ep[:, :].rearrange("q (h t) -> q h t", h=2)
                        nc.scalar.activation(epv[:, :, lo:512], spv[:, :, lo:512],
                                             EXP, scale=0.125)
                        for h in range(2):
                            nc.vector.tensor_mul(
                                epv[:, h, lo:lo + 128], epv[:, h, lo:lo + 128], tri_sb)
                    for h in range(2):
                        vcol = 65 * (2 * p + h)
                        if lo < 256:
                            nc.tensor.matmul(
                                avp[:, 256 * h + lo:256 * h + 256],
                                vt[c][:, vcol:vcol + 65],
                                ep[:, 512 * h + lo:512 * h + 256],
                                start=(c == 0), stop=(c == 13),
                            )
                        blo = max(lo, 256)
                        nc.tensor.matmul(
                            avp[:, 512 + 256 * h + blo - 256:512 + 256 * h + 256],
                            vt[c][:, vcol:vcol + 65],
                            ep[:, 512 * h + blo:512 * h + 512],
                            start=(c == 0), stop=(c == nch - 1),
                        )

                def emit_norm_half(p, tb, avp, half, fast=False):
                    # bank-local layout: h0 at [ob:ob+256], h1 at [ob+256:ob+512]
                    dma = nc.scalar if fast else nc.gpsimd
                    ob = 512 * half
                    W = 256
                    dh = nrm.tile([65, W], F32, name=f"dhh_{half}", tag="dhh")
                    tmpu = nrm.tile([65, W], F32, name=f"tuh_{half}", tag="tuh")
                    ao_raw = nrm.tile([128, W], F32, name=f"arh_{half}", tag="arh")
                    nc.vector.tensor_copy(dh[64:65, :], avp[64:65, ob:ob + W])
                    nc.vector.tensor_copy(tmpu, avp[0:65, ob + 256:ob + 512])
                    dscr = drp.tile([2, W], F32, name=f"dsh_{half}", tag="dsh")
                    dma.dma_start(out=dscr[0:1, :], in_=dh[64:65, :])
                    dma.dma_start(out=dscr[1:2, :], in_=tmpu[64:65, :])
                    nc.vector.tensor_copy(ao_raw[0:64, :], avp[0:64, ob:ob + W])
                    dma.dma_start(out=ao_raw[64:128, :], in_=tmpu[0:64, :])
                    bc = nrm.tile([128, W], F32, name=f"bch_{half}", tag="bch")
                    dma.dma_start(
                        out=bc[0:64, :], in_=dscr[0:1, :].partition_broadcast(64))
                    dma.dma_start(
                        out=bc[64:128, :], in_=dscr[1:2, :].partition_broadcast(64))
                    rc = nrm.tile([128, W], F32, name=f"rch_{half}", tag="rch")
                    nc.vector.reciprocal_approx_fast(out=rc, in_=bc)
                    tsl = slice(tb * 512 + 256 * half, tb * 512 + 256 * half + W)
                    nc.vector.tensor_mul(ao[p][:, tsl], ao_raw, rc)

                def emit_proj_group(tt, eb, tag="av", on_act=False):
                    ps = psA.tile([128, 512], F32, name=f"py_{tt}_{eb}", tag=tag)
                    for pp in range(NP):
                        nc.tensor.matmul(
                            ps, ao[pp][:, tt * 128:(tt + 1) * 128],
                            wo_r[:, pp, eb * 512:(eb + 1) * 512],
                            start=(pp == 0), stop=(pp == NP - 1),
                        )
                    ys = st.tile([128, 512], F16, name=f"ys_{tt}_{eb}", tag="ys", bufs=8)
                    if on_act:
                        nc.scalar.copy(ys, ps)
                    else:
                        nc.vector.tensor_copy(ys, ps)
                    nc.sync.dma_start(
                        out=y[tt * 128:(tt + 1) * 128, eb * 512:(eb + 1) * 512], in_=ys)

                # ---- v half 0 for the first t-block (pairs 0/1), plus two
                # extra groups to bridge the PE gap while ACT drains the
                # lead-in bias adds (keeps the HAM clock-gate warm) ----
                for tt in range(6):
                    emit_v_group(tt, 0)

                blocks = [(p, tb) for p in range(NP) for tb in range(NB)]
                bidx = {b: i for i, b in enumerate(blocks)}

                # window queue with deadlines: item = (deadline, kind, args);
                # deadline i = must be emitted before block i+1 starts.
                wq_items = []
                for tt in range(6, NC):
                    wq_items.append((bidx[(0, tt // 4)] - 1, "v", (tt, 0)))
                for tt in range(NC):
                    wq_items.append((bidx[(2, tt // 4)] - 1, "v", (tt, 1)))
                for pp in (1, 2, 3):
                    for tb in range(NB):
                        for nm in ("q", "k"):
                            wq_items.append(
                                (bidx[(pp, tb)] - 1, "qk", (pp, nm, tb)))
                wq_items.sort(key=lambda it: it[0])
                NODL = 999

                def emit_item(kind, args):
                    if kind == "qk":
                        emit_qk_group(*args)
                    elif kind == "v":
                        emit_v_group(*args)
                    else:
                        emit_proj_group(*args)

                def pop_window(n):
                    for _ in range(n):
                        if wq_items:
                            _, kind, args = wq_items.pop(0)
                            emit_item(kind, args)

                def drain_due(key):
                    while wq_items and wq_items[0][0] <= key:
                        _, kind, args = wq_items.pop(0)
                        emit_item(kind, args)

                pending = None
                pre_sc = None
                nsteps = []
                nmeta = None

                def norm_step():
                    nonlocal nsteps, nmeta
                    if nsteps:
                        nsteps.pop(0)()
                        if not nsteps and nmeta is not None:
                            np_, ntb_ = nmeta
                            nmeta = None
                            if np_ == NP - 1:
                                wq_items.extend(
                                    (NODL, "proj", (tt, eb))
                                    for tt in range(4 * ntb_, 4 * ntb_ + 4)
                                    for eb in range(2))

                for bi, (p, tb) in enumerate(blocks):
                    drain_due(bi)
                    nch = 4 * (tb + 1)
                    avp = psA.tile([65, 1024], F32, name=f"avp_{p}_{tb}", tag="av")
                    av0 = avp[:, 0:512]
                    av1 = avp[:, 512:1024]
                    sc = pre_sc if pre_sc is not None else emit_scores(p, tb, 0)
                    pre_sc = None
                    # emit the previous block's norm early (frees its PSUM slot
                    # for the interleaved groups); for short all-diagonal blocks
                    # wait until a couple of tri-multiplies are queued on DVE.
                    norm_c = min(3, nch - 1)
                    for c in range(nch):
                        if c + 1 < nch:
                            sc_next = emit_scores(p, tb, c + 1)
                        elif bi + 1 < len(blocks):
                            # cross-block lookahead: next block's first scores
                            pre_sc = emit_scores(blocks[bi + 1][0], blocks[bi + 1][1], 0)
                            sc_next = None
                        else:
                            sc_next = None
                        emit_expav(p, tb, c, sc, av0, av1, nch)
                        sc = sc_next
                        if c == norm_c and pending is not None:
                            nsteps = norm_thunks(*pending)
                            nmeta = (pending[0], pending[1])
                            pending = None
                        norm_step()
                        if c % 2 == 1:
                            pop_window(1)
                    while nsteps:
                        norm_step()
                    pending = (p, tb, avp)
                emit_norm(*pending, fast=True)
                wq_items.extend((NODL, "proj", (tt, eb))
                                for tt in range(12, 16) for eb in range(2))
                i = 0
                while wq_items:
                    _, kind, args = wq_items.pop(0)
                    if kind == "proj":
                        emit_proj_group(*args, tag=("s" if i % 2 else "av"), on_act=True)
                        i += 1
                    else:
                        emit_item(kind, args)

    nc.compile()
    return nc


def get_nc():
    if "nc" not in _cache:
        _cache["nc"] = _build_nc()
    return _cache["nc"]


def make_in_maps(x, w_qkv, b_qkv, w_out, b_out):
    """Per-core input dicts. Core = b*2 + g."""
    x = np.asarray(x, dtype=np.float32)
    w_qkv = np.asarray(w_qkv, dtype=np.float32)
    b_qkv = np.asarray(b_qkv, dtype=np.float32)
    w_out = np.asarray(w_out, dtype=np.float32)

    wq_full, wk_full, wv_full = w_qkv[:, 0:E], w_qkv[:, E:2 * E], w_qkv[:, 2 * E:3 * E]
    bq_full, bk_full, bv_full = b_qkv[0:E], b_qkv[E:2 * E], b_qkv[2 * E:3 * E]

    idx = np.arange(128)
    tri = (idx[:, None] <= idx[None, :]).astype(np.float16)  # tri[s,t]=1 iff s<=t

    in_maps = []
    for core in range(NCORES):
        b, g = core // 2, core % 2
        h0 = g * HL
        cols = slice(h0 * D, (h0 + HL) * D)
        wq_l = wq_full[:, cols]
        wk_l = wk_full[:, cols]
        wv_l = wv_full[:, cols]
        bq_l = bq_full[cols]
        bk_l = bk_full[cols]
        bv_l = bv_full[cols]

        wqk_s = np.empty((2 * NP, 128, NE, 128), dtype=np.float16)
        for p in range(NP):
            wqk_s[2 * p] = wq_l[:, p * 128:(p + 1) * 128].reshape(NE, 128, 128).transpose(1, 0, 2)
            wqk_s[2 * p + 1] = wk_l[:, p * 128:(p + 1) * 128].reshape(NE, 128, 128).transpose(1, 0, 2)

        wv2 = np.zeros((E, VW), dtype=np.float16)
        bv2 = np.zeros((1, VW), dtype=np.float16)
        for h in range(HL):
            wv2[:, h * 65:h * 65 + 64] = wv_l[:, h * 64:(h + 1) * 64].astype(np.float16)
            bv2[0, h * 65:h * 65 + 64] = bv_l[h * 64:(h + 1) * 64].astype(np.float16)
            bv2[0, h * 65 + 64] = 1.0

        bcol = np.zeros((128, 2 * NP), dtype=np.float32)
        for p in range(NP):
            bcol[:, 2 * p] = bq_l[p * 128:(p + 1) * 128]
            bcol[:, 2 * p + 1] = bk_l[p * 128:(p + 1) * 128]

        wv2d = wv2.reshape(NE, 128, 2, VW // 2).transpose(2, 1, 0, 3)
        wo_l = w_out[g * EL:(g + 1) * EL, :].astype(np.float16)
        wo2 = wo_l.reshape(NP, 128, E).transpose(1, 0, 2)
        in_maps.append({
            "xT": np.ascontiguousarray(x[b].T.astype(np.float16)),
            "wq0d": np.ascontiguousarray(wqk_s[0]),
            "wk0d": np.ascontiguousarray(wqk_s[1]),
            "wqkr": np.ascontiguousarray(wqk_s[2:8].transpose(1, 0, 2, 3)),
            "wv2d": np.ascontiguousarray(wv2d),
            "wod": np.ascontiguousarray(wo2),
            "rowsd": bv2,
            "bcold": bcol,
            "trid": tri,
        })
    return in_maps


def gather_output(results, b_out):
    out = np.empty((B, T, E), dtype=np.float32)
    b_out = np.asarray(b_out, dtype=np.float32)
    for b in range(B):
        out[b] = (results[2 * b]["y"].astype(np.float32)
                  + results[2 * b + 1]["y"].astype(np.float32) + b_out[None, :])
    return out


def kernel(x, w_qkv, b_qkv, w_out, b_out):
    from concourse.bass_utils import run_bass_kernel_spmd

    nc = get_nc()
    in_maps = make_in_maps(x, w_qkv, b_qkv, w_out, b_out)
    r = run_bass_kernel_spmd(nc, in_maps, core_ids=list(range(NCORES)))
    return gather_output(r.results, np.asarray(b_out, dtype=np.float32))


# revision 81
# speedup vs baseline: 1.0140x; 1.0099x over previous
"""Causal self-attention (B=4, T=2048, E=1024, H=16, D=64) on 8 TRN2 NeuronCores.

Sharding: core = b*2 + g  (data parallel over batch b in 0..3, tensor parallel
over head-halves g in 0..1; 8 local heads per core, column-split QKV /
row-split out projection). Host sums the two partial out-projections per batch
and adds b_out.

Device kernel (per core). All matmuls run with fp16 operands (1 cycle/row on
the PE) accumulating in fp32 PSUM; activations/weights are pre-cast to fp16 on
the host so they DMA straight into their SBUF tiles:
  - qT/kT [128 = 2 heads x 64, T] per head-pair; v' [T, 8 x (64 v-dims + ones
    col)]; the ones column makes the attn@v matmul emit softmax denominators.
  - transposed-scores attention per (pair, t-block of 512): scoresT[s,t]
    chunks via row-tiled K=64 matmul pairs into a 2-bank PSUM tile, one exp
    per chunk on ACT (both heads, scale=1/8 folded in), causal diagonal via
    in-place [128,128] triangle multiplies on DVE, av accumulated over
    s-chunks with causal width narrowing.
  - PE kept dense (HAM warm) while ACT grinds exps: remaining qkv-projection
    groups and out-projection groups are fed through the PSUM slot freed by
    each block's normalization.
  - normalization off the PE path: denominator rows bounce through DRAM and
    partition-broadcast back by DMA, one-shot reciprocal_approx_fast on DVE,
    multiplied into the fp16 attention output.
  - lead-in: pair-0 q/k run as an e-major accumulation wave across 8 PSUM
    banks so the PE consumes each x chunk as its DMA lands; qkv/out-projection
    fill work is drip-fed one group per two attention chunks from a
    deadline-sorted queue; y is written back as fp16 and summed on the host.
"""
import numpy as np

B, T, E, H, D = 4, 2048, 1024, 16, 64
HL = H // 2           # local heads per core (8)
NP = HL // 2          # head pairs per core (4)
EL = HL * D           # local attn-out width (512)
VW = HL * (D + 1)     # v' width with ones columns (520)
NCORES = 8
NB = T // 512         # t-blocks (4)
NC = T // 128         # s-chunks (16)
NE = E // 128         # e-chunks (8)

_cache = {}


def _build_nc():
    import concourse.bacc as bacc
    import concourse.mybir as mybir
    from concourse.tile import TileContext

    F32 = mybir.dt.float32
    F16 = mybir.dt.float16
    EXP = mybir.ActivationFunctionType.Exp

    nc = bacc.Bacc(None, target_bir_lowering=False)
    xT = nc.dram_tensor("xT", [E, T], F16, kind="ExternalInput")
    wq0d = nc.dram_tensor("wq0d", [128, NE, 128], F16, kind="ExternalInput")
    wk0d = nc.dram_tensor("wk0d", [128, NE, 128], F16, kind="ExternalInput")
    wqkr = nc.dram_tensor("wqkr", [128, 6, NE, 128], F16, kind="ExternalInput")
    wv2d = nc.dram_tensor("wv2d", [2, 128, NE, VW // 2], F16, kind="ExternalInput")
    wod = nc.dram_tensor("wod", [128, NP, E], F16, kind="ExternalInput")
    rowsd = nc.dram_tensor("rowsd", [1, VW], F16, kind="ExternalInput")   # bv2
    bcold = nc.dram_tensor("bcold", [128, 2 * NP], F32, kind="ExternalInput")
    trid = nc.dram_tensor("trid", [128, 128], F16, kind="ExternalInput")
    y = nc.dram_tensor("y", [T, E], F16, kind="ExternalOutput")

    with TileContext(nc) as tc:
        with (
            tc.tile_pool(name="const", bufs=1) as cpool,
            tc.tile_pool(name="p_keep", bufs=1) as keep,
            tc.tile_pool(name="p_st", bufs=2) as st,
        ):
            # ---- long-lived fp16 tensors, DMA'd directly (priority order) ----
            HALF = VW // 2  # 260
            xt = [keep.tile([128, T], F16, name=f"xt{e}", tag=f"xt{e}") for e in range(NE)]
            wr = {}
            wr[(0, "q")] = keep.tile([128, NE, 128], F16, name="wq0", tag="wq0")
            wr[(0, "k")] = keep.tile([128, NE, 128], F16, name="wk0", tag="wk0")
            wr_rest = keep.tile([128, 6, NE, 128], F16, name="wrest", tag="wrest")
            for p in range(1, NP):
                for i, nm in enumerate(("q", "k")):
                    wr[(p, nm)] = wr_rest[:, 2 * (p - 1) + i]
            wv_r = [keep.tile([128, NE, HALF], F16, name=f"wv{h_}", tag=f"wv{h_}")
                    for h_ in range(2)]
            # priority: pair-0 q/k weights (scalar queue) race with x chunks
            # (sync); bulk weights follow x on the sync queue so they don't
            # steal HBM bandwidth from the critical path.
            nc.scalar.dma_start(out=wr[(0, "q")], in_=wq0d[:, :, :])
            nc.sync.dma_start(out=xt[0][:, 0:1024], in_=xT[0:128, 0:1024])
            nc.sync.dma_start(out=xt[0][:, 1024:2048], in_=xT[0:128, 1024:2048])
            for e in range(1, NE):
                nc.sync.dma_start(out=xt[e], in_=xT[e * 128:(e + 1) * 128, :])
            nc.scalar.dma_start(out=wr[(0, "k")], in_=wk0d[:, :, :])
            nc.sync.dma_start(out=wv_r[0], in_=wv2d[0])
            nc.sync.dma_start(out=wr_rest[:, 0:2], in_=wqkr[:, 0:2])
            nc.sync.dma_start(out=wv_r[1], in_=wv2d[1])
            nc.sync.dma_start(out=wr_rest[:, 2:6], in_=wqkr[:, 2:6])
            wo_r = keep.tile([128, NP, E], F16, name="wo_r")
            nc.sync.dma_start(out=wo_r, in_=wod[:, :, :])
            # ---- constants ----
            tri_sb = cpool.tile([128, 128], F16, name="tri_sb")
            nc.scalar.dma_start(out=tri_sb, in_=trid[:, :])
            bcol = cpool.tile([128, 2 * NP], F32, name="bcol")
            nc.scalar.dma_start(out=bcol, in_=bcold[:, :])
            ones_r = cpool.tile([1, 512], F16, name="ones_r")
            nc.vector.memset(ones_r, 1.0)
            bv_r = cpool.tile([1, VW], F16, name="bv_r")
            nc.scalar.dma_start(out=bv_r, in_=rowsd[:, :])
            # preload the ACT exp table during the lead-in
            warm = cpool.tile([1, 16], F32, name="warm")
            nc.scalar.activation(warm, tri_sb[0:1, 0:16], EXP, scale=0.125)
            qt = [keep.tile([128, T], F16, name=f"qt{p}", tag=f"qt{p}") for p in range(NP)]
            kt = [keep.tile([128, T], F16, name=f"kt{p}", tag=f"kt{p}") for p in range(NP)]
            vt = [keep.tile([128, VW], F16, name=f"vt{t_}", tag=f"vt{t_}") for t_ in range(NC)]
            ao = [keep.tile([128, T], F16, name=f"ao{p}", tag=f"ao{p}") for p in range(NP)]

            # ---- lead-in: pair-0 q/k via an e-major accumulation wave so the
            # PE consumes each x chunk as it lands (8 open PSUM groups) ----
            with tc.tile_pool(name="psL", bufs=1, space="PSUM") as psL:
                lead = []
                for i, (nm, tb) in enumerate(
                        [(nm, tb) for tb in range(NB) for nm in ("q", "k")]):
                    lead.append((nm, tb, psL.tile(
                        [128, 512], F32, name=f"pl{i}", tag=f"g{i}")))
                for e in range(NE - 1):
                    for nm, tb, ps in lead:
                        nc.tensor.matmul(
                            ps, wr[(0, nm)][:, e, :],
                            xt[e][:, tb * 512:(tb + 1) * 512],
                            start=(e == 0), stop=False,
                        )
                # last e-chunk per group, bias-add right after so the adds
                # overlap the remaining groups' matmuls (tb=0 first so the
                # first attention block unblocks early)
                for nm, tb, ps in lead:
                    nc.tensor.matmul(
                        ps, wr[(0, nm)][:, NE - 1, :],
                        xt[NE - 1][:, tb * 512:(tb + 1) * 512],
                        start=False, stop=True,
                    )
                    dst = qt[0] if nm == "q" else kt[0]
                    col = 0 if nm == "q" else 1
                    nc.scalar.add(dst[:, tb * 512:(tb + 1) * 512], ps,
                                  bcol[:, col:col + 1])

            with (
                tc.tile_pool(name="p_att", bufs=6) as att,
                tc.tile_pool(name="p_nrm", bufs=3) as nrm,
                tc.tile_pool(name="p_dr", bufs=2, space="DRAM") as drp,
                tc.tile_pool(name="psA", bufs=2, space="PSUM") as psA,
            ):
                def emit_qk_group(p, nm, tb, on_act=False):
                    dst = qt[p] if nm == "q" else kt[p]
                    ps = psA.tile([128, 512], F32, name=f"ps{nm}_{p}_{tb}", tag="av")
                    for e in range(NE):
                        nc.tensor.matmul(
                            ps, wr[(p, nm)][:, e, :], xt[e][:, tb * 512:(tb + 1) * 512],
                            start=(e == 0), stop=(e == NE - 1),
                        )
                    col = 2 * p + (0 if nm == "q" else 1)
                    dsl = dst[:, tb * 512:(tb + 1) * 512]
                    if on_act:
                        nc.scalar.add(dsl, ps, bcol[:, col:col + 1])
                    else:
                        nc.vector.tensor_scalar_add(dsl, ps, bcol[:, col:col + 1])

                def emit_v_group(tt, h_):
                    ps = psA.tile([128, HALF], F32, name=f"psv_{tt}_{h_}", tag="av")
                    for e in range(NE):
                        nc.tensor.matmul(
                            ps, xt[e][:, tt * 128:(tt + 1) * 128], wv_r[h_][:, e, :],
                            start=(e == 0), stop=False,
                        )
                    nc.tensor.matmul(
                        ps, ones_r[:, 0:128], bv_r[:, h_ * HALF:(h_ + 1) * HALF],
                        start=False, stop=True,
                    )
                    dsl = vt[tt][:, h_ * HALF:(h_ + 1) * HALF]
                    nc.vector.tensor_copy(dsl, ps)

                def emit_scores(p, tb, c):
                    j = c - 4 * tb
                    lo = 128 * j if j >= 0 else 0
                    sp = psA.tile([128, 1024], F32, name=f"s_{p}_{tb}_{c}", tag="s")
                    nc.tensor.matmul(
                        sp[:, lo:512], kt[p][0:64, c * 128:(c + 1) * 128],
                        qt[p][0:64, tb * 512 + lo:(tb + 1) * 512],
                        start=True, stop=True, tile_position=(0, 0),
                    )
                    nc.tensor.matmul(
                        sp[:, 512 + lo:1024], kt[p][64:128, c * 128:(c + 1) * 128],
                        qt[p][64:128, tb * 512 + lo:(tb + 1) * 512],
                        start=True, stop=True, tile_position=(64, 0),
                    )
                    return sp, lo, j

                def emit_expav(p, tb, c, sc, av0, av1, nch):
                    sp, lo, j = sc
                    ep = att.tile([128, 1024], F16, name=f"e_{p}_{tb}_{c}", tag="ep")
                    if j < 0:
                        nc.scalar.activation(ep, sp, EXP, scale=0.125)
                    else:
                        spv = sp[:, :].rearrange("q (h t) -> q h t", h=2)
                        epv = ep[:, :].rearrange("q (h t) -> q h t", h=2)
                        nc.scalar.activation(epv[:, :, lo:512], spv[:, :, lo:512],
                                             EXP, scale=0.125)
                        for h in range(2):
                            nc.vector.tensor_mul(
                                epv[:, h, lo:lo + 128], epv[:, h, lo:lo + 128], tri_sb)
                    for h, av in ((0, av0), (1, av1)):
                        vcol = 65 * (2 * p + h)
                        nc.tensor.matmul(
                            av[:, lo:512], vt[c][:, vcol:vcol + 65],
                            ep[:, 512 * h + lo:512 * h + 512],
                            start=(c == 0), stop=(c == nch - 1),
                        )

                def norm_thunks(p, tb, avp, fast=False):
                    # normalization as ~0.7us DVE pieces, dripped one per chunk
                    # so diagonal tri-multiplies interleave instead of queueing
                    # behind a 4us burst. fast=True (final block): the bounce
                    # DMAs ride the HWDGE scalar queue (~0.6us first byte)
                    # instead of the gpsimd SWDGE queue (~1-2us).
                    dma = nc.scalar if fast else nc.gpsimd
                    av0 = avp[:, 0:512]
                    av1 = avp[:, 512:1024]
                    dh = nrm.tile([65, 512], F32, name=f"dh_{p}_{tb}", tag="dh")
                    tmpu = nrm.tile([65, 512], F32, name=f"tu_{p}_{tb}", tag="tu")
                    ao_raw = nrm.tile([128, 512], F32, name=f"ar_{p}_{tb}", tag="ar")
                    dscr = drp.tile([2, 512], F32, name=f"ds_{p}_{tb}", tag="ds")
                    bc = nrm.tile([128, 512], F32, name=f"bc_{p}_{tb}", tag="bc")
                    rc = nrm.tile([128, 512], F32, name=f"rc_{p}_{tb}", tag="rc")
                    tbsl = slice(tb * 512, (tb + 1) * 512)

                    def s1():
                        nc.vector.tensor_copy(dh[64:65, :], av0[64:65, :])
                        dma.dma_start(out=dscr[0:1, :], in_=dh[64:65, :])

                    def s2():
                        nc.vector.tensor_copy(tmpu, av1[0:65, :])
                        dma.dma_start(out=dscr[1:2, :], in_=tmpu[64:65, :])
                        dma.dma_start(out=ao_raw[64:128, :], in_=tmpu[0:64, :])
                        dma.dma_start(
                            out=bc[0:64, :],
                            in_=dscr[0:1, :].partition_broadcast(64))
                        dma.dma_start(
                            out=bc[64:128, :],
                            in_=dscr[1:2, :].partition_broadcast(64))

                    def s3():
                        nc.vector.tensor_copy(ao_raw[0:64, :], av0[0:64, :])

                    def s4():
                        nc.vector.reciprocal_approx_fast(out=rc, in_=bc)

                    def s5():
                        nc.vector.tensor_mul(ao[p][:, tbsl], ao_raw, rc)

                    return [s1, s2, s3, s4, s5]

                def emit_norm(p, tb, avp, fast=False):
                    for s in norm_thunks(p, tb, avp, fast=fast):
                        s()

                def emit_expav_split(p, tb, c, sc, avp, nch):
                    # final block: av accumulation split at the t-midpoint with
                    # each t-half in its OWN PSUM bank (one open accumulation
                    # group per bank). Bank A (cols 0:512) = both heads'
                    # t[0:256), closes at chunk 13; bank B (cols 512:1024) =
                    # t[256:512), closes at the last chunk.
                    sp, lo, j = sc
                    ep = att.tile([128, 1024], F16, name=f"e_{p}_{tb}_{c}", tag="ep")
                    if j < 0:
                        nc.scalar.activation(ep, sp, EXP, scale=0.125)
                    else:
                        spv = sp[:, :].rearrange("q (h t) -> q h t", h=2)
                        epv = ep[:, :].rearrange("q (h t) -> q h t", h=2)
                        nc.scalar.activation(epv[:, :, lo:512], spv[:, :, lo:512],
                                             EXP, scale=0.125)
                        for h in range(2):
                            nc.vector.tensor_mul(
                                epv[:, h, lo:lo + 128], epv[:, h, lo:lo + 128], tri_sb)
                    for h in range(2):
                        vcol = 65 * (2 * p + h)
                        if lo < 256:
                            nc.tensor.matmul(
                                avp[:, 256 * h + lo:256 * h + 256],
                                vt[c][:, vcol:vcol + 65],
                                ep[:, 512 * h + lo:512 * h + 256],
                                start=(c == 0), stop=(c == 13),
                            )
                        blo = max(lo, 256)
                        nc.tensor.matmul(
                            avp[:, 512 + 256 * h + blo - 256:512 + 256 * h + 256],
                            vt[c][:, vcol:vcol + 65],
                            ep[:, 512 * h + blo:512 * h + 512],
                            start=(c == 0), stop=(c == nch - 1),
                        )

                def emit_norm_half(p, tb, avp, half, fast=False):
                    # bank-local layout: h0 at [ob:ob+256], h1 at [ob+256:ob+512]
                    dma = nc.scalar if fast else nc.gpsimd
                    ob = 512 * half
                    W = 256
                    dh = nrm.tile([65, W], F32, name=f"dhh_{half}", tag="dhh")
                    tmpu = nrm.tile([65, W], F32, name=f"tuh_{half}", tag="tuh")
                    ao_raw = nrm.tile([128, W], F32, name=f"arh_{half}", tag="arh")
                    nc.vector.tensor_copy(dh[64:65, :], avp[64:65, ob:ob + W])
                    nc.vector.tensor_copy(tmpu, avp[0:65, ob + 256:ob + 512])
                    dscr = drp.tile([2, W], F32, name=f"dsh_{half}", tag="dsh")
                    dma.dma_start(out=dscr[0:1, :], in_=dh[64:65, :])
                    dma.dma_start(out=dscr[1:2, :], in_=tmpu[64:65, :])
                    nc.vector.tensor_copy(ao_raw[0:64, :], avp[0:64, ob:ob + W])
                    dma.dma_start(out=ao_raw[64:128, :], in_=tmpu[0:64, :])
                    bc = nrm.tile([128, W], F32, name=f"bch_{half}", tag="bch")
                    dma.dma_start(
                        out=bc[0:64, :], in_=dscr[0:1, :].partition_broadcast(64))
                    dma.dma_start(
                        out=bc[64:128, :], in_=dscr[1:2, :].partition_broadcast(64))
                    rc = nrm.tile([128, W], F32, name=f"rch_{half}", tag="rch")
                    nc.vector.reciprocal_approx_fast(out=rc, in_=bc)
                    tsl = slice(tb * 512 + 256 * half, tb * 512 + 256 * half + W)
                    nc.vector.tensor_mul(ao[p][:, tsl], ao_raw, rc)

                def emit_proj_group(tt, eb, tag="av", on_act=False):
                    ps = psA.tile([128, 512], F32, name=f"py_{tt}_{eb}", tag=tag)
                    for pp in range(NP):
                        nc.tensor.matmul(
                            ps, ao[pp][:, tt * 128:(tt + 1) * 128],
                            wo_r[:, pp, eb * 512:(eb + 1) * 512],
                            start=(pp == 0), stop=(pp == NP - 1),
                        )
                    ys = st.tile([128, 512], F16, name=f"ys_{tt}_{eb}", tag="ys", bufs=8)
                    if on_act:
                        nc.scalar.copy(ys, ps)
                    else:
                        nc.vector.tensor_copy(ys, ps)
                    nc.sync.dma_start(
                        out=y[tt * 128:(tt + 1) * 128, eb * 512:(eb + 1) * 512], in_=ys)

                # ---- v half 0 for the first t-block (pairs 0/1), plus two
                # extra groups to bridge the PE gap while ACT drains the
                # lead-in bias adds (keeps the HAM clock-gate warm) ----
                for tt in range(6):
                    emit_v_group(tt, 0)

                blocks = [(p, tb) for p in range(NP) for tb in range(NB)]
                bidx = {b: i for i, b in enumerate(blocks)}

                # window queue with deadlines: item = (deadline, kind, args);
                # deadline i = must be emitted before block i+1 starts.
                wq_items = []
                for tt in range(6, NC):
                    wq_items.append((bidx[(0, tt // 4)] - 1, "v", (tt, 0)))
                for tt in range(NC):
                    wq_items.append((bidx[(2, tt // 4)] - 1, "v", (tt, 1)))
                for pp in (1, 2, 3):
                    for tb in range(NB):
                        for nm in ("q", "k"):
                            wq_items.append(
                                (bidx[(pp, tb)] - 1, "qk", (pp, nm, tb)))
                wq_items.sort(key=lambda it: it[0])
                NODL = 999

                def emit_item(kind, args):
                    if kind == "qk":
                        emit_qk_group(*args)
                    elif kind == "v":
                        emit_v_group(*args)
                    else:
                        emit_proj_group(*args)

                def pop_window(n):
                    for _ in range(n):
                        if wq_items:
                            _, kind, args = wq_items.pop(0)
                            emit_item(kind, args)

                def drain_due(key):
                    while wq_items and wq_items[0][0] <= key:
                        _, kind, args = wq_items.pop(0)
                        emit_item(kind, args)

                pending = None
                pre_sc = None
                nsteps = []
                nmeta = None

                def norm_step():
                    nonlocal nsteps, nmeta
                    if nsteps:
                        nsteps.pop(0)()
                        if not nsteps and nmeta is not None:
                            np_, ntb_ = nmeta
                            nmeta = None
                            if np_ == NP - 1:
                                wq_items.extend(
                                    (NODL, "proj", (tt, eb))
                                    for tt in range(4 * ntb_, 4 * ntb_ + 4)
                                    for eb in range(2))

                for bi, (p, tb) in enumerate(blocks):
                    drain_due(bi)
                    nch = 4 * (tb + 1)
                    avp = psA.tile([65, 1024], F32, name=f"avp_{p}_{tb}", tag="av")
                    av0 = avp[:, 0:512]
                    av1 = avp[:, 512:1024]
                    sc = pre_sc if pre_sc is not None else emit_scores(p, tb, 0)
                    pre_sc = None
                    # emit the previous block's norm early (frees its PSUM slot
                    # for the interleaved groups); for short all-diagonal blocks
                    # wait until a couple of tri-multiplies are queued on DVE.
                    norm_c = 1 if tb > 0 else min(2, nch - 1)
                    for c in range(nch):
                        if c + 1 < nch:
                            sc_next = emit_scores(p, tb, c + 1)
                        elif bi + 1 < len(blocks):
                            # cross-block lookahead: next block's first scores
                            pre_sc = emit_scores(blocks[bi + 1][0], blocks[bi + 1][1], 0)
                            sc_next = None
                        else:
                            sc_next = None
                        emit_expav(p, tb, c, sc, av0, av1, nch)
                        sc = sc_next
                        if c == norm_c and pending is not None:
                            nsteps = norm_thunks(*pending)
                            nmeta = (pending[0], pending[1])
                            pending = None
                        norm_step()
                        if c % 2 == 1:
                            pop_window(1)
                    while nsteps:
                        norm_step()
                    pending = (p, tb, avp)
                emit_norm(*pending, fast=True)
                wq_items.extend((NODL, "proj", (tt, eb))
                                for tt in range(12, 16) for eb in range(2))
                i = 0
                while wq_items:
                    _, kind, args = wq_items.pop(0)
                    if kind == "proj":
                        emit_proj_group(*args, tag=("s" if i % 2 else "av"), on_act=True)
                        i += 1
                    else:
                        emit_item(kind, args)

    nc.compile()
    return nc


def get_nc():
    if "nc" not in _cache:
        _cache["nc"] = _build_nc()
    return _cache["nc"]


def make_in_maps(x, w_qkv, b_qkv, w_out, b_out):
    """Per-core input dicts. Core = b*2 + g."""
    x = np.asarray(x, dtype=np.float32)
    w_qkv = np.asarray(w_qkv, dtype=np.float32)
    b_qkv = np.asarray(b_qkv, dtype=np.float32)
    w_out = np.asarray(w_out, dtype=np.float32)

    wq_full, wk_full, wv_full = w_qkv[:, 0:E], w_qkv[:, E:2 * E], w_qkv[:, 2 * E:3 * E]
    bq_full, bk_full, bv_full = b_qkv[0:E], b_qkv[E:2 * E], b_qkv[2 * E:3 * E]

    idx = np.arange(128)
    tri = (idx[:, None] <= idx[None, :]).astype(np.float16)  # tri[s,t]=1 iff s<=t

    in_maps = []
    for core in range(NCORES):
        b, g = core // 2, core % 2
        h0 = g * HL
        cols = slice(h0 * D, (h0 + HL) * D)
        wq_l = wq_full[:, cols]
        wk_l = wk_full[:, cols]
        wv_l = wv_full[:, cols]
        bq_l = bq_full[cols]
        bk_l = bk_full[cols]
        bv_l = bv_full[cols]

        wqk_s = np.empty((2 * NP, 128, NE, 128), dtype=np.float16)
        for p in range(NP):
            wqk_s[2 * p] = wq_l[:, p * 128:(p + 1) * 128].reshape(NE, 128, 128).transpose(1, 0, 2)
            wqk_s[2 * p + 1] = wk_l[:, p * 128:(p + 1) * 128].reshape(NE, 128, 128).transpose(1, 0, 2)

        wv2 = np.zeros((E, VW), dtype=np.float16)
        bv2 = np.zeros((1, VW), dtype=np.float16)
        for h in range(HL):
            wv2[:, h * 65:h * 65 + 64] = wv_l[:, h * 64:(h + 1) * 64].astype(np.float16)
            bv2[0, h * 65:h * 65 + 64] = bv_l[h * 64:(h + 1) * 64].astype(np.float16)
            bv2[0, h * 65 + 64] = 1.0

        bcol = np.zeros((128, 2 * NP), dtype=np.float32)
        for p in range(NP):
            bcol[:, 2 * p] = bq_l[p * 128:(p + 1) * 128]
            bcol[:, 2 * p + 1] = bk_l[p * 128:(p + 1) * 128]

        wv2d = wv2.reshape(NE, 128, 2, VW // 2).transpose(2, 1, 0, 3)
        wo_l = w_out[g * EL:(g + 1) * EL, :].astype(np.float16)
        wo2 = wo_l.reshape(NP, 128, E).transpose(1, 0, 2)
        in_maps.append({
            "xT": np.ascontiguousarray(x[b].T.astype(np.float16)),
            "wq0d": np.ascontiguousarray(wqk_s[0]),
            "wk0d": np.ascontiguousarray(wqk_s[1]),
            "wqkr": np.ascontiguousarray(wqk_s[2:8].transpose(1, 0, 2, 3)),
            "wv2d": np.ascontiguousarray(wv2d),
            "wod": np.ascontiguousarray(wo2),
            "rowsd": bv2,
            "bcold": bcol,
            "trid": tri,
        })
    return in_maps


def gather_output(results, b_out):
    out = np.empty((B, T, E), dtype=np.float32)
    b_out = np.asarray(b_out, dtype=np.float32)
    for b in range(B):
        out[b] = (results[2 * b]["y"].astype(np.float32)
                  + results[2 * b + 1]["y"].astype(np.float32) + b_out[None, :])
    return out


def kernel(x, w_qkv, b_qkv, w_out, b_out):
    from concourse.bass_utils import run_bass_kernel_spmd

    nc = get_nc()
    in_maps = make_in_maps(x, w_qkv, b_qkv, w_out, b_out)
    r = run_bass_kernel_spmd(nc, in_maps, core_ids=list(range(NCORES)))
    return gather_output(r.results, np.asarray(b_out, dtype=np.float32))
